# revision 38
# baseline (speedup 1.0000x reference)
"""GRU (ragged sequences) Trainium2 Bass kernel — chunked-Picard v2b.

The GRU is solved per time-chunk by Picard iteration (the step map is
strongly contractive), with the h-recurrence solved exactly along the
chunk by one tensor_tensor_scan per sweep:

  sweep s (gates from the previous iterate's trajectory, wide over t):
    s_g  = W_g_hh h_prev[t-1] + W_g_ih x_t + b_g    (PE, f32r, psum accum)
    r, z = sigmoid(s_rz)                            (Act)
    pre  = s_n_ih + r * (W_n_hh h_prev[t-1] + bhn)  (DVE stt + PE accum)
    n    = tanh(pre)                                (Act)
    h_t  = z_t h_{t-1} + (1-z_t) n_t                (exact affine scan, DVE)

v2b vs v1:
  * gi is RECOMPUTED on PE each sweep (Wih x accumulated into the same
    psum group as Whh h) instead of precomputed + evacuated to SBUF:
    kills all three PSUM->SBUF evacuation ops per chunk (DVE was the
    bottleneck engine) at the cost of PE matmuls (PE has headroom).
  * Sweep schedule (rzn, zn, rzn, zn): the r gate is only recomputed on
    sweeps 0 and 2 (rel err 9.9e-3 vs 7.9e-3 for full, budget 2e-2).
  * Variable-width chunk plans per slot: the last chunk of each slot is
    trimmed to the slot's max sequence length (rounded up to 64, min 256
    to keep f32r matmuls at 1 cycle/row): 23 -> 20.5 chunk-equivalents.
  * Ragged masking via host-side x poisoning: for t >= seq_len, x[:,t]
    is replaced by v solving W_z_ih v + b_ih_z = 40, so z saturates to
    exactly 1.0 in fp32 and h freezes bit-exactly.  Kills the mask row
    DMA and the per-chunk mask matmul.
  * Output tail (t >= slot plan end) filled on host from the last column
    instead of on-device broadcast+DMA.

Sequences are sorted by length and interleaved across cores (core c gets
ranks c, c+8, ...) so all cores share one live pattern / one program.
x is host-pretransposed to [B, I, T]; output is [B, H, T].
"""

import sys
import numpy as np

sys.path.insert(0, "/opt/trn_rl_repo")

B, T_FULL, I, H = 64, 2048, 128, 128
NCORES = 8
BC = B // NCORES          # sequences per core
KMAX = 512
SCHED = ("rzn", "zn", "rzn", "zn")
USE_SWEEP0 = False   # per-partition gh0 sweep-0 form: measured slower (queue hops)

_CACHE = {}


def _plan_slot(maxlen, T):
    """Chunks of 512 plus a trimmed tail in [256, 512] rounded up to 64."""
    plan = []
    t0 = 0
    while t0 + KMAX <= maxlen:
        plan.append((t0, KMAX))
        t0 += KMAX
    rem = maxlen - t0
    if rem > 0:
        w = min(KMAX, max(256, -(-rem // 64) * 64))
        w = min(w, T - t0)
        plan.append((t0, w))
    return tuple(plan)


def _assignment(seq_len, T):
    """Interleaved sorted assignment: core c, slot p <- rank p*NCORES + c."""
    sl = np.asarray(seq_len)
    order = np.argsort(-sl, kind="stable")
    perm = order.reshape(BC, NCORES)           # [slot, core]
    plans = tuple(_plan_slot(int(sl[perm[p]].max()), T) for p in range(BC))
    return perm, plans


def _build(T, plans):
    from contextlib import ExitStack
    import concourse.bacc as bacc
    import concourse.mybir as mybir
    import concourse.tile as tile

    f32 = mybir.dt.float32
    f32r = mybir.dt.float32r
    Alu = mybir.AluOpType
    Act = mybir.ActivationFunctionType

    nrounds = max(len(p) for p in plans)

    nc = bacc.Bacc("TRN2", target_bir_lowering=False, debug=False,
                   num_devices=NCORES)

    xt = nc.dram_tensor("xt", [BC, I, T], f32r, kind="ExternalInput").ap()
    wih3 = nc.dram_tensor("wih3", [I, 3 * H], f32r, kind="ExternalInput").ap()
    whh3 = nc.dram_tensor("whh3", [H, 3 * H], f32r, kind="ExternalInput").ap()
    # per-gate total biases as 1-row weights: r,z: b_ih+b_hh, n: b_ih only,
    # row 3: b_hh_n (for the sweep-0 gh0 trick)
    gibt = nc.dram_tensor("gibt", [4, 128], f32r, kind="ExternalInput").ap()
    # bias cols: 0: b_hh_n (t1 scalar), 1: -b_ih_n (tanh bias), 2: b_z total
    bcol = nc.dram_tensor("bcol", [H, 3], f32, kind="ExternalInput").ap()
    onesd = nc.dram_tensor("onesd", [1, KMAX], f32r, kind="ExternalInput").ap()
    yt = nc.dram_tensor("yt", [BC, H, T], f32r, kind="ExternalOutput").ap()

    with tile.TileContext(nc) as tc, ExitStack() as ctx:
        const = ctx.enter_context(tc.tile_pool(name="const", bufs=1))
        xpool = ctx.enter_context(tc.tile_pool(name="x", bufs=2))
        hppool = ctx.enter_context(tc.tile_pool(name="hp", bufs=2))
        rzpool = ctx.enter_context(tc.tile_pool(name="rz", bufs=1))
        npool = ctx.enter_context(tc.tile_pool(name="nn", bufs=8))
        unpool = ctx.enter_context(tc.tile_pool(name="un", bufs=8))
        zcpool = ctx.enter_context(tc.tile_pool(name="zc", bufs=8))
        ghpool = ctx.enter_context(tc.tile_pool(name="gh0", bufs=2))
        ps_rz = ctx.enter_context(tc.tile_pool(name="ps_rz", bufs=4, space="PSUM"))

        wih_sb = const.tile([128, 3 * H], f32r, tag="wih")
        nc.sync.dma_start(out=wih_sb[:], in_=wih3)
        whh_sb = const.tile([128, 3 * H], f32r, tag="whh")
        nc.sync.dma_start(out=whh_sb[:], in_=whh3)
        gib_rows = []
        for g in range(4):
            row = const.tile([1, 128], f32r, tag=f"gib{g}", name=f"gib{g}")
            nc.sync.dma_start(out=row[:], in_=gibt[g:g + 1, :])
            gib_rows.append(row)
        bcol_sb = const.tile([128, 3], f32, tag="bcol")
        nc.sync.dma_start(out=bcol_sb[:], in_=bcol)
        ones_sb = const.tile([1, KMAX], f32r, tag="ones")
        nc.sync.dma_start(out=ones_sb[:], in_=onesd)
        zero_e = const.tile([128, 2], f32, tag="zeroe")
        nc.vector.memset(zero_e[:], 0.0)
        zero_er = const.tile([128, 2], f32r, tag="zeroer")
        nc.vector.tensor_copy(out=zero_er[:], in_=zero_e[:])
        brc_sb = const.tile([128, KMAX], f32, tag="brc")
        nc.vector.memset(brc_sb[:], 0.0)

        # entry: f32 view (scan initial / scalar operands) + f32r 2-col view
        # (matmul data; 1-col matmuls fail the ISA check) of the previous
        # chunk's final h
        entry = {b: zero_e[:, 0:1] for b in range(BC)}
        entry_r = {b: zero_er[:, 0:2] for b in range(BC)}
        hps, xs, rzs = {}, {}, {}

        def preamble(b, t0, K):
            xtile = xpool.tile([128, KMAX], f32r, tag=f"x{b}", name=f"x{b}")
            nc.sync.dma_start(out=xtile[:, 0:K], in_=xt[b, :, t0:t0 + K])
            xs[b] = xtile
            # hp trajectory tile: col 0 = h_entry, cols 1..K = h_1..h_K.
            hp = hppool.tile([128, KMAX + 1], f32r, tag=f"hp{b}", name=f"hp{b}")
            if USE_SWEEP0:
                # sweep-0 gh0 form needs no broadcast; only col 0 must hold
                # the entry for later sweeps' matmuls.
                nc.gpsimd.tensor_copy(out=hp[:, 0:1], in_=entry[b])
            else:
                # sweep-0 guess: h_prev[t] = h_entry for all t (brc as zero
                # shape-donor: no false dep); alternate engine by slot.
                eng = nc.gpsimd if b % 2 == 0 else nc.vector
                eng.tensor_scalar(out=hp[:, 0:K], in0=brc_sb[:, 0:K],
                                  scalar1=0.0, scalar2=entry[b],
                                  op0=Alu.mult, op1=Alu.add)
            hps[b] = hp
            rzs[b] = rzpool.tile([128, 2 * KMAX], f32, tag=f"rz{b}",
                                 name=f"rz{b}")

        def sweep0(b, t0, K, un_on_pool):
            """Sweep 0: the trajectory guess is the constant h_entry, so
            W_hh h collapses to per-partition scalars gh0 = W_hh h_entry + b
            computed by six 1-column matmuls; the wide matmuls are only the
            three W_ih x products and the gate biases ride activation bias
            APs / the stt scalar."""
            hp, xtile, rz = hps[b], xs[b], rzs[b]
            prz = ps_rz.tile([128, 2 * KMAX], f32, tag="przn")
            Z0 = KMAX
            # gh0: 2-col matmuls (1-col matmuls fail the ISA check); only
            # the second output column of each pair is meaningful.
            # pair g: col 2g+1 = W_g e + bias_g  (g=2 bias is b_hh_n)
            for g in range(3):
                nc.tensor.matmul(prz[:, 2 * g:2 * g + 2],
                                 whh_sb[:, g * 128:(g + 1) * 128],
                                 entry_r[b], start=True, stop=False)
                nc.tensor.matmul(prz[:, 2 * g:2 * g + 2],
                                 gib_rows[g if g < 2 else 3][0:1, :],
                                 ones_sb[0:1, 0:2], start=False, stop=True)
            gh0 = ghpool.tile([128, 6], f32, tag="gh0", name="gh0")
            nc.vector.tensor_copy(out=gh0[:], in_=prz[:, 0:6])
            # wide input projections (gi) — overwrite the pe0 columns
            nc.tensor.matmul(prz[:, 0:K], wih_sb[:, 0:128], xtile[:, 0:K],
                             start=True, stop=True, skip_group_check=True)
            nc.tensor.matmul(prz[:, Z0:Z0 + K], wih_sb[:, 128:256],
                             xtile[:, 0:K], start=True, stop=True)
            nc.scalar.activation(rz[:, 0:K], prz[:, 0:K], Act.Sigmoid,
                                 bias=gh0[:, 1:2])
            nc.scalar.activation(rz[:, Z0:Z0 + K], prz[:, Z0:Z0 + K],
                                 Act.Sigmoid, bias=gh0[:, 3:4])
            # gi_n into the (dead) r region
            nc.tensor.matmul(prz[:, 0:K], wih_sb[:, 256:384], xtile[:, 0:K],
                             start=True, stop=True, skip_group_check=True)
            # pre' = r * gh0_n + gi_n  (b_ih_n rides the tanh bias)
            nc.vector.scalar_tensor_tensor(
                out=prz[:, Z0:Z0 + K], in0=rz[:, 0:K], scalar=gh0[:, 5:6],
                in1=prz[:, 0:K], op0=Alu.mult, op1=Alu.add)
            nsb = npool.tile([128, KMAX], f32, tag="nn", name="nsb")
            nc.scalar.activation(nsb[:, 0:K], prz[:, Z0:Z0 + K],
                                 Act.Tanh, scale=-1.0, bias=bcol_sb[:, 1:2])
            un = unpool.tile([128, KMAX], f32, tag="un", name="un")
            if un_on_pool:
                zc = zcpool.tile([128, KMAX], f32, tag="zc", name="zc")
                nc.gpsimd.tensor_scalar(out=zc[:, 0:K],
                                        in0=rz[:, Z0:Z0 + K], scalar1=1.0,
                                        scalar2=None, op0=Alu.subtract)
                nc.gpsimd.tensor_tensor(out=un[:, 0:K], in0=zc[:, 0:K],
                                        in1=nsb[:, 0:K], op=Alu.mult)
            else:
                nc.vector.scalar_tensor_tensor(
                    out=un[:, 0:K], in0=rz[:, Z0:Z0 + K], scalar=1.0,
                    in1=nsb[:, 0:K], op0=Alu.subtract, op1=Alu.mult)
            nc.vector.tensor_tensor_scan(
                out=hp[:, 1:K + 1], data0=rz[:, Z0:Z0 + K],
                data1=un[:, 0:K], initial=entry[b],
                op0=Alu.mult, op1=Alu.add)

        def sweep(b, gates, un_on_pool, prz, of, w, init_ap):
            """One sweep over columns [of, of+w) of slot b's current chunk.
            Splitting a sweep into column pieces (thin rounds) shortens the
            serial chain: stage s of piece j overlaps stage s+1 of piece
            j-1.  The scan chains across pieces via init_ap."""
            hp, xtile, rz = hps[b], xs[b], rzs[b]
            Z0 = KMAX
            lo, hi = of, of + w          # r region / trajectory window
            zlo, zhi = Z0 + of, Z0 + of + w   # z region window
            if "r" in gates:
                # r group with the bias as a 1-row matmul (merged rz sigmoid
                # can't take per-gate bias APs)
                nc.tensor.matmul(prz[:, lo:hi], whh_sb[:, 0:128],
                                 hp[:, lo:hi], start=True, stop=False)
                nc.tensor.matmul(prz[:, lo:hi], wih_sb[:, 0:128],
                                 xtile[:, lo:hi], start=False, stop=False)
                nc.tensor.matmul(prz[:, lo:hi], gib_rows[0][0:1, :],
                                 ones_sb[0:1, 0:w], start=False, stop=True)
                nc.tensor.matmul(prz[:, zlo:zhi], whh_sb[:, 128:256],
                                 hp[:, lo:hi], start=True, stop=False)
                nc.tensor.matmul(prz[:, zlo:zhi], wih_sb[:, 128:256],
                                 xtile[:, lo:hi], start=False, stop=False)
                nc.tensor.matmul(prz[:, zlo:zhi], gib_rows[1][0:1, :],
                                 ones_sb[0:1, 0:w], start=False, stop=True)
                prz3 = prz.rearrange("p (g k) -> p g k", g=2)
                rz3 = rz.rearrange("p (g k) -> p g k", g=2)
                nc.scalar.activation(rz3[:, :, lo:hi], prz3[:, :, lo:hi],
                                     Act.Sigmoid)
            else:
                # z-only: bias rides the sigmoid's per-partition bias AP
                nc.tensor.matmul(prz[:, zlo:zhi], whh_sb[:, 128:256],
                                 hp[:, lo:hi], start=True, stop=False)
                nc.tensor.matmul(prz[:, zlo:zhi], wih_sb[:, 128:256],
                                 xtile[:, lo:hi], start=False, stop=True)
                nc.scalar.activation(rz[:, zlo:zhi], prz[:, zlo:zhi],
                                     Act.Sigmoid, bias=bcol_sb[:, 2:3])
            # ghn into the (dead or unused) r psum region
            nc.tensor.matmul(prz[:, lo:hi], whh_sb[:, 256:384], hp[:, lo:hi],
                             start=True, stop=True, skip_group_check=True)
            # t1 = (ghn + bhn) * r  -> overwrite dead s_z psum region
            nc.vector.scalar_tensor_tensor(
                out=prz[:, zlo:zhi], in0=prz[:, lo:hi],
                scalar=bcol_sb[:, 0:1],
                in1=rz[:, lo:hi], op0=Alu.add, op1=Alu.mult)
            # pre' = t1 + W_n_ih x: PE accumulates onto t1 in-place (psum
            # has_written bits from the s_z matmuls survive the DVE
            # overwrite, so start=False adds).  b_ih_n rides the tanh bias.
            nc.tensor.matmul(prz[:, zlo:zhi], wih_sb[:, 256:384],
                             xtile[:, lo:hi], start=False, stop=True,
                             skip_group_check=True)
            # nneg = tanh(-(pre' + b_ih_n)) = -n  (negation via scale,
            # b_ih_n via the per-partition bias AP: bias col 1 = -b_ih_n)
            nsb = npool.tile([128, KMAX], f32, tag="nn", name="nsb")
            nc.scalar.activation(nsb[:, 0:w], prz[:, zlo:zhi],
                                 Act.Tanh, scale=-1.0, bias=bcol_sb[:, 1:2])
            # un = (z-1)*(-n) = (1-z)*n
            un = unpool.tile([128, KMAX], f32, tag="un", name="un")
            if un_on_pool:
                # GpSimd path (SBUF-only): zc = z-1, then un = zc * nneg
                zc = zcpool.tile([128, KMAX], f32, tag="zc", name="zc")
                nc.gpsimd.tensor_scalar(out=zc[:, 0:w],
                                        in0=rz[:, zlo:zhi], scalar1=1.0,
                                        scalar2=None, op0=Alu.subtract)
                nc.gpsimd.tensor_tensor(out=un[:, 0:w], in0=zc[:, 0:w],
                                        in1=nsb[:, 0:w], op=Alu.mult)
            else:
                nc.vector.scalar_tensor_tensor(
                    out=un[:, 0:w], in0=rz[:, zlo:zhi], scalar=1.0,
                    in1=nsb[:, 0:w], op0=Alu.subtract, op1=Alu.mult)
            # exact affine solve along the piece: h_t = z_t h_{t-1} + un_t
            nc.vector.tensor_tensor_scan(
                out=hp[:, lo + 1:hi + 1], data0=rz[:, zlo:zhi],
                data1=un[:, 0:w], initial=init_ap,
                op0=Alu.mult, op1=Alu.add)

        def finish(b, t0, K):
            hp = hps[b]
            nc.sync.dma_start(out=yt[b, :, t0:t0 + K], in_=hp[:, 1:K + 1])
            entry[b] = hp[:, K:K + 1].bitcast(f32)
            entry_r[b] = hp[:, K - 1:K + 1]

        for ci in range(nrounds):
            livebs = [b for b in range(BC) if len(plans[b]) > ci]
            for b in livebs:
                t0, K = plans[b][ci]
                preamble(b, t0, K)
            # thin rounds are latency-bound: split each sweep into column
            # pieces so successive stages pipeline across pieces.
            npieces = 2 if len(livebs) <= 3 else 1
            for s, gates in enumerate(SCHED):
                przs = {}
                for b in livebs:
                    przs[b] = ps_rz.tile([128, 2 * KMAX], f32, tag="przn",
                                         name="przn")
                for j in range(npieces):
                    for b in livebs:
                        t0, K = plans[b][ci]
                        of = K * j // npieces
                        w = K * (j + 1) // npieces - of
                        init = (entry[b] if j == 0
                                else hps[b][:, of:of + 1].bitcast(f32))
                        # alternate un's engine by slot within each sweep
                        # phase so DVE and Pool balance inside every phase;
                        # thin rounds keep un on DVE (latency-bound).
                        unp = len(livebs) >= 5 and (b + s) % 2 == 0
                        sweep(b, gates, unp, przs[b], of, w, init)
                        if s == len(SCHED) - 1 and j == npieces - 1:
                            finish(b, t0, K)

    nc.compile()
    return nc


def _host_prep(x, seq_len, w_ih, w_hh, b_ih, b_hh, perm):
    T = x.shape[1]
    x = np.asarray(x, np.float32)
    w_ih = np.asarray(w_ih, np.float32)
    w_hh = np.asarray(w_hh, np.float32)
    b_ih = np.asarray(b_ih, np.float32)
    b_hh = np.asarray(b_hh, np.float32)
    seq_len = np.asarray(seq_len).astype(np.int64)
    xt_all = np.ascontiguousarray(x.transpose(0, 2, 1))  # [B, I, T]
    # Poison columns t >= seq_len so that gi_z + b_ih_z ~= 60: z saturates
    # to exactly 1.0 in fp32 (gh_z is bounded by ~6) and h freezes
    # bit-exactly, reproducing the reference's frozen outputs past seq_len.
    # Truncated-SVD solve: tiny singular directions of W_z_ih are dropped so
    # that ||v|| stays small enough for the PE's reduced-precision f32r
    # accumulation (a full solve can give ||v|| ~ 1e6 on an ill-conditioned
    # W_z and f32r noise ~1e3 destroys the freeze).  Dropping sigma_i only
    # perturbs s_z by ~ +-c|u_i^T 1||u_i| << c, still far above saturation.
    Wz = w_ih[H:2 * H].astype(np.float64)
    c = np.full(H, 60.0) - b_ih[H:2 * H].astype(np.float64)
    U, S, Vt = np.linalg.svd(Wz)
    Sinv = np.where(S >= S.max() / 300.0, 1.0 / S, 0.0)
    v = (Vt.T @ (Sinv * (U.T @ c))).astype(np.float32)
    for b in range(B):
        if seq_len[b] < T:
            xt_all[b, :, seq_len[b]:] = v[:, None]
    wih3 = np.ascontiguousarray(w_ih.T)
    whh3 = np.ascontiguousarray(w_hh.T)
    gibt = np.stack([
        b_ih[0:H] + b_hh[0:H],
        b_ih[H:2 * H] + b_hh[H:2 * H],
        b_ih[2 * H:],
        b_hh[2 * H:],
    ], axis=0).astype(np.float32)
    bcol_v = np.stack([
        b_hh[2 * H:],                          # t1 stt scalar (b_hh_n)
        -b_ih[2 * H:],                         # tanh bias (-b_ih_n)
        b_ih[H:2 * H] + b_hh[H:2 * H],         # zn-sweep sigmoid bias (b_z)
    ], axis=1).astype(np.float32)
    in_maps = []
    for c in range(NCORES):
        idx = perm[:, c]                       # slot p -> original seq index
        in_maps.append({
            "xt": np.ascontiguousarray(xt_all[idx]),
            "wih3": wih3, "whh3": whh3, "gibt": gibt, "bcol": bcol_v,
            "onesd": np.ones((1, KMAX), np.float32),
        })
    return in_maps


LAST_RESULTS = None


def kernel(x, seq_len, w_ih, w_hh, b_ih, b_hh):
    global LAST_RESULTS
    from concourse import bass_utils
    T = x.shape[1]
    perm, plans = _assignment(seq_len, T)
    key = (T, plans)
    if key not in _CACHE:
        _CACHE[key] = _build(T, plans)
    nc = _CACHE[key]
    in_maps = _host_prep(np.asarray(x), np.asarray(seq_len), np.asarray(w_ih),
                         np.asarray(w_hh), np.asarray(b_ih), np.asarray(b_hh),
                         perm)
    res = bass_utils.run_bass_kernel_spmd(nc, in_maps,
                                          core_ids=list(range(NCORES)))
    LAST_RESULTS = res
    y = np.empty((B, T, H), np.float32)
    for c in range(NCORES):
        ytc = np.array(res.results[c]["yt"])   # [BC, H, T]
        for p in range(BC):
            t0, K = plans[p][-1]
            t_end = t0 + K
            if t_end < T:
                # past the slot's plan end, h is frozen: replicate last col
                ytc[p, :, t_end:] = ytc[p, :, t_end - 1][:, None]
        y[perm[:, c]] = ytc.transpose(0, 2, 1)
    return np.ascontiguousarray(y)


# revision 43
# speedup vs baseline: 1.0906x; 1.0906x over previous
"""GRU (ragged sequences) Trainium2 Bass kernel — chunked-Picard v2b.

The GRU is solved per time-chunk by Picard iteration (the step map is
strongly contractive), with the h-recurrence solved exactly along the
chunk by one tensor_tensor_scan per sweep:

  sweep s (gates from the previous iterate's trajectory, wide over t):
    s_g  = W_g_hh h_prev[t-1] + W_g_ih x_t + b_g    (PE, f32r, psum accum)
    r, z = sigmoid(s_rz)                            (Act)
    pre  = s_n_ih + r * (W_n_hh h_prev[t-1] + bhn)  (DVE stt + PE accum)
    n    = tanh(pre)                                (Act)
    h_t  = z_t h_{t-1} + (1-z_t) n_t                (exact affine scan, DVE)

v2b vs v1:
  * gi is RECOMPUTED on PE each sweep (Wih x accumulated into the same
    psum group as Whh h) instead of precomputed + evacuated to SBUF:
    kills all three PSUM->SBUF evacuation ops per chunk (DVE was the
    bottleneck engine) at the cost of PE matmuls (PE has headroom).
  * Sweep schedule (rzn, zn, rzn, zn): the r gate is only recomputed on
    sweeps 0 and 2 (rel err 9.9e-3 vs 7.9e-3 for full, budget 2e-2).
  * Variable-width chunk plans per slot: the last chunk of each slot is
    trimmed to the slot's max sequence length (rounded up to 64, min 256
    to keep f32r matmuls at 1 cycle/row): 23 -> 20.5 chunk-equivalents.
  * Ragged masking via host-side x poisoning: for t >= seq_len, x[:,t]
    is replaced by v solving W_z_ih v + b_ih_z = 40, so z saturates to
    exactly 1.0 in fp32 and h freezes bit-exactly.  Kills the mask row
    DMA and the per-chunk mask matmul.
  * Output tail (t >= slot plan end) filled on host from the last column
    instead of on-device broadcast+DMA.

Sequences are sorted by length and interleaved across cores (core c gets
ranks c, c+8, ...) so all cores share one live pattern / one program.
x is host-pretransposed to [B, I, T]; output is [B, H, T].
"""

import sys
import numpy as np

sys.path.insert(0, "/opt/trn_rl_repo")

B, T_FULL, I, H = 64, 2048, 128, 128
NCORES = 8
BC = B // NCORES          # sequences per core
KMAX = 512
SCHED = ("rzn", "zn", "rzn", "zn")
USE_SWEEP0 = False   # per-partition gh0 sweep-0 form: measured slower (queue hops)

_CACHE = {}


def _plan_slot(maxlen, T):
    """Chunks of 512 plus a trimmed tail in [256, 512] rounded up to 64."""
    plan = []
    t0 = 0
    while t0 + KMAX <= maxlen:
        plan.append((t0, KMAX))
        t0 += KMAX
    rem = maxlen - t0
    if rem > 0:
        w = min(KMAX, max(256, -(-rem // 64) * 64))
        w = min(w, T - t0)
        plan.append((t0, w))
    return tuple(plan)


def _assignment(seq_len, T):
    """Interleaved sorted assignment: core c, slot p <- rank p*NCORES + c."""
    sl = np.asarray(seq_len)
    order = np.argsort(-sl, kind="stable")
    perm = order.reshape(BC, NCORES)           # [slot, core]
    plans = tuple(_plan_slot(int(sl[perm[p]].max()), T) for p in range(BC))
    return perm, plans


def _chains(plans, G=6):
    """Pack the BC slot-groups into G serial chains with near-equal chunk
    counts (greedy LPT) so every round keeps >=G-1 independent chains alive
    (the tail rounds of the plain per-slot layout starve the engines).
    h resets to zero at group boundaries inside a chain.
    Returns: list of chains; each chain is a list of
    (group p, t0, K, is_group_start) chunk records."""
    order = sorted(range(len(plans)), key=lambda p: -len(plans[p]))
    chains = [[] for _ in range(G)]
    loads = [0] * G
    for p in order:
        i = loads.index(min(loads))
        chains[i].append(p)
        loads[i] += len(plans[p])
    out = []
    for groups in chains:
        recs = []
        for p in groups:
            for k, (t0, K) in enumerate(plans[p]):
                recs.append((p, t0, K, k == 0))
        out.append(recs)
    # longest chains first so the final round's chains start earliest
    out.sort(key=len, reverse=True)
    return out


def _build(T, plans):
    from contextlib import ExitStack
    import concourse.bacc as bacc
    import concourse.mybir as mybir
    import concourse.tile as tile

    f32 = mybir.dt.float32
    f32r = mybir.dt.float32r
    Alu = mybir.AluOpType
    Act = mybir.ActivationFunctionType

    nc = bacc.Bacc("TRN2", target_bir_lowering=False, debug=False,
                   num_devices=NCORES)

    xt = nc.dram_tensor("xt", [BC, I, T], f32r, kind="ExternalInput").ap()
    wih3 = nc.dram_tensor("wih3", [I, 3 * H], f32r, kind="ExternalInput").ap()
    whh3 = nc.dram_tensor("whh3", [H, 3 * H], f32r, kind="ExternalInput").ap()
    # per-gate total biases as 1-row weights: r,z: b_ih+b_hh, n: b_ih only,
    # row 3: b_hh_n (for the sweep-0 gh0 trick)
    gibt = nc.dram_tensor("gibt", [4, 128], f32r, kind="ExternalInput").ap()
    # bias cols: 0: b_hh_n (t1 scalar), 1: -b_ih_n (tanh bias), 2: b_z total
    bcol = nc.dram_tensor("bcol", [H, 3], f32, kind="ExternalInput").ap()
    onesd = nc.dram_tensor("onesd", [1, KMAX], f32r, kind="ExternalInput").ap()
    yt = nc.dram_tensor("yt", [BC, H, T], f32r, kind="ExternalOutput").ap()

    with tile.TileContext(nc) as tc, ExitStack() as ctx:
        const = ctx.enter_context(tc.tile_pool(name="const", bufs=1))
        xpool = ctx.enter_context(tc.tile_pool(name="x", bufs=2))
        hppool = ctx.enter_context(tc.tile_pool(name="hp", bufs=2))
        rzpool = ctx.enter_context(tc.tile_pool(name="rz", bufs=1))
        npool = ctx.enter_context(tc.tile_pool(name="nn", bufs=8))
        unpool = ctx.enter_context(tc.tile_pool(name="un", bufs=8))
        zcpool = ctx.enter_context(tc.tile_pool(name="zc", bufs=8))
        ghpool = ctx.enter_context(tc.tile_pool(name="gh0", bufs=2))
        ps_rz = ctx.enter_context(tc.tile_pool(name="ps_rz", bufs=4, space="PSUM"))

        wih_sb = const.tile([128, 3 * H], f32r, tag="wih")
        nc.sync.dma_start(out=wih_sb[:], in_=wih3)
        whh_sb = const.tile([128, 3 * H], f32r, tag="whh")
        nc.sync.dma_start(out=whh_sb[:], in_=whh3)
        gib_rows = []
        for g in range(4):
            row = const.tile([1, 128], f32r, tag=f"gib{g}", name=f"gib{g}")
            nc.sync.dma_start(out=row[:], in_=gibt[g:g + 1, :])
            gib_rows.append(row)
        bcol_sb = const.tile([128, 3], f32, tag="bcol")
        nc.sync.dma_start(out=bcol_sb[:], in_=bcol)
        ones_sb = const.tile([1, KMAX], f32r, tag="ones")
        nc.sync.dma_start(out=ones_sb[:], in_=onesd)
        zero_e = const.tile([128, 2], f32, tag="zeroe")
        nc.vector.memset(zero_e[:], 0.0)
        zero_er = const.tile([128, 2], f32r, tag="zeroer")
        nc.vector.tensor_copy(out=zero_er[:], in_=zero_e[:])
        brc_sb = const.tile([128, KMAX], f32, tag="brc")
        nc.vector.memset(brc_sb[:], 0.0)

        # entry: f32 view (scan initial / scalar operands) + f32r 2-col view
        # (matmul data; 1-col matmuls fail the ISA check) of the previous
        # chunk's final h
        entry = {b: zero_e[:, 0:1] for b in range(BC)}
        entry_r = {b: zero_er[:, 0:2] for b in range(BC)}
        hps, xs, rzs = {}, {}, {}

        def preamble(b, p, t0, K, reset):
            if reset:
                # first chunk of a new sequence-group in this chain
                entry[b] = zero_e[:, 0:1]
                entry_r[b] = zero_er[:, 0:2]
            xtile = xpool.tile([128, KMAX], f32r, tag=f"x{b}", name=f"x{b}")
            nc.sync.dma_start(out=xtile[:, 0:K], in_=xt[p, :, t0:t0 + K])
            xs[b] = xtile
            # hp trajectory tile: col 0 = h_entry, cols 1..K = h_1..h_K.
            hp = hppool.tile([128, KMAX + 1], f32r, tag=f"hp{b}", name=f"hp{b}")
            if USE_SWEEP0:
                # sweep-0 gh0 form needs no broadcast; only col 0 must hold
                # the entry for later sweeps' matmuls.
                nc.gpsimd.tensor_copy(out=hp[:, 0:1], in_=entry[b])
            else:
                # sweep-0 guess: h_prev[t] = h_entry for all t (brc as zero
                # shape-donor: no false dep); alternate engine by slot.
                eng = nc.gpsimd if b % 2 == 0 else nc.vector
                eng.tensor_scalar(out=hp[:, 0:K], in0=brc_sb[:, 0:K],
                                  scalar1=0.0, scalar2=entry[b],
                                  op0=Alu.mult, op1=Alu.add)
            hps[b] = hp
            rzs[b] = rzpool.tile([128, 2 * KMAX], f32, tag=f"rz{b}",
                                 name=f"rz{b}")

        def sweep0(b, t0, K, un_on_pool):
            """Sweep 0: the trajectory guess is the constant h_entry, so
            W_hh h collapses to per-partition scalars gh0 = W_hh h_entry + b
            computed by six 1-column matmuls; the wide matmuls are only the
            three W_ih x products and the gate biases ride activation bias
            APs / the stt scalar."""
            hp, xtile, rz = hps[b], xs[b], rzs[b]
            prz = ps_rz.tile([128, 2 * KMAX], f32, tag="przn")
            Z0 = KMAX
            # gh0: 2-col matmuls (1-col matmuls fail the ISA check); only
            # the second output column of each pair is meaningful.
            # pair g: col 2g+1 = W_g e + bias_g  (g=2 bias is b_hh_n)
            for g in range(3):
                nc.tensor.matmul(prz[:, 2 * g:2 * g + 2],
                                 whh_sb[:, g * 128:(g + 1) * 128],
                                 entry_r[b], start=True, stop=False)
                nc.tensor.matmul(prz[:, 2 * g:2 * g + 2],
                                 gib_rows[g if g < 2 else 3][0:1, :],
                                 ones_sb[0:1, 0:2], start=False, stop=True)
            gh0 = ghpool.tile([128, 6], f32, tag="gh0", name="gh0")
            nc.vector.tensor_copy(out=gh0[:], in_=prz[:, 0:6])
            # wide input projections (gi) — overwrite the pe0 columns
            nc.tensor.matmul(prz[:, 0:K], wih_sb[:, 0:128], xtile[:, 0:K],
                             start=True, stop=True, skip_group_check=True)
            nc.tensor.matmul(prz[:, Z0:Z0 + K], wih_sb[:, 128:256],
                             xtile[:, 0:K], start=True, stop=True)
            nc.scalar.activation(rz[:, 0:K], prz[:, 0:K], Act.Sigmoid,
                                 bias=gh0[:, 1:2])
            nc.scalar.activation(rz[:, Z0:Z0 + K], prz[:, Z0:Z0 + K],
                                 Act.Sigmoid, bias=gh0[:, 3:4])
            # gi_n into the (dead) r region
            nc.tensor.matmul(prz[:, 0:K], wih_sb[:, 256:384], xtile[:, 0:K],
                             start=True, stop=True, skip_group_check=True)
            # pre' = r * gh0_n + gi_n  (b_ih_n rides the tanh bias)
            nc.vector.scalar_tensor_tensor(
                out=prz[:, Z0:Z0 + K], in0=rz[:, 0:K], scalar=gh0[:, 5:6],
                in1=prz[:, 0:K], op0=Alu.mult, op1=Alu.add)
            nsb = npool.tile([128, KMAX], f32, tag="nn", name="nsb")
            nc.scalar.activation(nsb[:, 0:K], prz[:, Z0:Z0 + K],
                                 Act.Tanh, scale=-1.0, bias=bcol_sb[:, 1:2])
            un = unpool.tile([128, KMAX], f32, tag="un", name="un")
            if un_on_pool:
                zc = zcpool.tile([128, KMAX], f32, tag="zc", name="zc")
                nc.gpsimd.tensor_scalar(out=zc[:, 0:K],
                                        in0=rz[:, Z0:Z0 + K], scalar1=1.0,
                                        scalar2=None, op0=Alu.subtract)
                nc.gpsimd.tensor_tensor(out=un[:, 0:K], in0=zc[:, 0:K],
                                        in1=nsb[:, 0:K], op=Alu.mult)
            else:
                nc.vector.scalar_tensor_tensor(
                    out=un[:, 0:K], in0=rz[:, Z0:Z0 + K], scalar=1.0,
                    in1=nsb[:, 0:K], op0=Alu.subtract, op1=Alu.mult)
            nc.vector.tensor_tensor_scan(
                out=hp[:, 1:K + 1], data0=rz[:, Z0:Z0 + K],
                data1=un[:, 0:K], initial=entry[b],
                op0=Alu.mult, op1=Alu.add)

        def sweep(b, gates, un_on_pool, prz, of, w, init_ap):
            """One sweep over columns [of, of+w) of slot b's current chunk.
            Splitting a sweep into column pieces (thin rounds) shortens the
            serial chain: stage s of piece j overlaps stage s+1 of piece
            j-1.  The scan chains across pieces via init_ap."""
            hp, xtile, rz = hps[b], xs[b], rzs[b]
            Z0 = KMAX
            lo, hi = of, of + w          # r region / trajectory window
            zlo, zhi = Z0 + of, Z0 + of + w   # z region window
            if "r" in gates:
                # r group with the bias as a 1-row matmul (merged rz sigmoid
                # can't take per-gate bias APs)
                nc.tensor.matmul(prz[:, lo:hi], whh_sb[:, 0:128],
                                 hp[:, lo:hi], start=True, stop=False)
                nc.tensor.matmul(prz[:, lo:hi], wih_sb[:, 0:128],
                                 xtile[:, lo:hi], start=False, stop=False)
                nc.tensor.matmul(prz[:, lo:hi], gib_rows[0][0:1, :],
                                 ones_sb[0:1, 0:w], start=False, stop=True)
                nc.tensor.matmul(prz[:, zlo:zhi], whh_sb[:, 128:256],
                                 hp[:, lo:hi], start=True, stop=False)
                nc.tensor.matmul(prz[:, zlo:zhi], wih_sb[:, 128:256],
                                 xtile[:, lo:hi], start=False, stop=False)
                nc.tensor.matmul(prz[:, zlo:zhi], gib_rows[1][0:1, :],
                                 ones_sb[0:1, 0:w], start=False, stop=True)
                prz3 = prz.rearrange("p (g k) -> p g k", g=2)
                rz3 = rz.rearrange("p (g k) -> p g k", g=2)
                nc.scalar.activation(rz3[:, :, lo:hi], prz3[:, :, lo:hi],
                                     Act.Sigmoid)
            else:
                # z-only: bias rides the sigmoid's per-partition bias AP
                nc.tensor.matmul(prz[:, zlo:zhi], whh_sb[:, 128:256],
                                 hp[:, lo:hi], start=True, stop=False)
                nc.tensor.matmul(prz[:, zlo:zhi], wih_sb[:, 128:256],
                                 xtile[:, lo:hi], start=False, stop=True)
                nc.scalar.activation(rz[:, zlo:zhi], prz[:, zlo:zhi],
                                     Act.Sigmoid, bias=bcol_sb[:, 2:3])
            # ghn into the (dead or unused) r psum region
            nc.tensor.matmul(prz[:, lo:hi], whh_sb[:, 256:384], hp[:, lo:hi],
                             start=True, stop=True, skip_group_check=True)
            # t1 = (ghn + bhn) * r  -> overwrite dead s_z psum region
            nc.vector.scalar_tensor_tensor(
                out=prz[:, zlo:zhi], in0=prz[:, lo:hi],
                scalar=bcol_sb[:, 0:1],
                in1=rz[:, lo:hi], op0=Alu.add, op1=Alu.mult)
            # pre' = t1 + W_n_ih x: PE accumulates onto t1 in-place (psum
            # has_written bits from the s_z matmuls survive the DVE
            # overwrite, so start=False adds).  b_ih_n rides the tanh bias.
            nc.tensor.matmul(prz[:, zlo:zhi], wih_sb[:, 256:384],
                             xtile[:, lo:hi], start=False, stop=True,
                             skip_group_check=True)
            # nneg = tanh(-(pre' + b_ih_n)) = -n  (negation via scale,
            # b_ih_n via the per-partition bias AP: bias col 1 = -b_ih_n)
            nsb = npool.tile([128, KMAX], f32, tag="nn", name="nsb")
            nc.scalar.activation(nsb[:, 0:w], prz[:, zlo:zhi],
                                 Act.Tanh, scale=-1.0, bias=bcol_sb[:, 1:2])
            # un = (z-1)*(-n) = (1-z)*n
            un = unpool.tile([128, KMAX], f32, tag="un", name="un")
            if un_on_pool:
                # GpSimd path (SBUF-only): zc = z-1, then un = zc * nneg
                zc = zcpool.tile([128, KMAX], f32, tag="zc", name="zc")
                nc.gpsimd.tensor_scalar(out=zc[:, 0:w],
                                        in0=rz[:, zlo:zhi], scalar1=1.0,
                                        scalar2=None, op0=Alu.subtract)
                nc.gpsimd.tensor_tensor(out=un[:, 0:w], in0=zc[:, 0:w],
                                        in1=nsb[:, 0:w], op=Alu.mult)
            else:
                nc.vector.scalar_tensor_tensor(
                    out=un[:, 0:w], in0=rz[:, zlo:zhi], scalar=1.0,
                    in1=nsb[:, 0:w], op0=Alu.subtract, op1=Alu.mult)
            # exact affine solve along the piece: h_t = z_t h_{t-1} + un_t
            nc.vector.tensor_tensor_scan(
                out=hp[:, lo + 1:hi + 1], data0=rz[:, zlo:zhi],
                data1=un[:, 0:w], initial=init_ap,
                op0=Alu.mult, op1=Alu.add)

        def finish(b, p, t0, K):
            hp = hps[b]
            nc.sync.dma_start(out=yt[p, :, t0:t0 + K], in_=hp[:, 1:K + 1])
            entry[b] = hp[:, K:K + 1].bitcast(f32)
            entry_r[b] = hp[:, K - 1:K + 1]

        chains = _chains(plans)
        nrounds = max(len(c) for c in chains)
        for ci in range(nrounds):
            livebs = [b for b in range(len(chains)) if ci < len(chains[b])]
            for b in livebs:
                p, t0, K, reset = chains[b][ci]
                preamble(b, p, t0, K, reset)
            for s, gates in enumerate(SCHED):
                przs = {}
                for b in livebs:
                    przs[b] = ps_rz.tile([128, 2 * KMAX], f32, tag="przn",
                                         name="przn")
                for b in livebs:
                    p, t0, K, reset = chains[b][ci]
                    # alternate un's engine by slot within each sweep
                    # phase so DVE and Pool balance inside every phase;
                    # thin rounds keep un on DVE (latency-bound).
                    unp = len(livebs) >= 5 and (b + s) % 2 == 0
                    sweep(b, gates, unp, przs[b], 0, K, entry[b])
                    if s == len(SCHED) - 1:
                        finish(b, p, t0, K)

    nc.compile()
    return nc


def _host_prep(x, seq_len, w_ih, w_hh, b_ih, b_hh, perm):
    T = x.shape[1]
    x = np.asarray(x, np.float32)
    w_ih = np.asarray(w_ih, np.float32)
    w_hh = np.asarray(w_hh, np.float32)
    b_ih = np.asarray(b_ih, np.float32)
    b_hh = np.asarray(b_hh, np.float32)
    seq_len = np.asarray(seq_len).astype(np.int64)
    xt_all = np.ascontiguousarray(x.transpose(0, 2, 1))  # [B, I, T]
    # Poison columns t >= seq_len so that gi_z + b_ih_z ~= 60: z saturates
    # to exactly 1.0 in fp32 (gh_z is bounded by ~6) and h freezes
    # bit-exactly, reproducing the reference's frozen outputs past seq_len.
    # Truncated-SVD solve: tiny singular directions of W_z_ih are dropped so
    # that ||v|| stays small enough for the PE's reduced-precision f32r
    # accumulation (a full solve can give ||v|| ~ 1e6 on an ill-conditioned
    # W_z and f32r noise ~1e3 destroys the freeze).  Dropping sigma_i only
    # perturbs s_z by ~ +-c|u_i^T 1||u_i| << c, still far above saturation.
    Wz = w_ih[H:2 * H].astype(np.float64)
    c = np.full(H, 60.0) - b_ih[H:2 * H].astype(np.float64)
    U, S, Vt = np.linalg.svd(Wz)
    Sinv = np.where(S >= S.max() / 300.0, 1.0 / S, 0.0)
    v = (Vt.T @ (Sinv * (U.T @ c))).astype(np.float32)
    for b in range(B):
        if seq_len[b] < T:
            xt_all[b, :, seq_len[b]:] = v[:, None]
    wih3 = np.ascontiguousarray(w_ih.T)
    whh3 = np.ascontiguousarray(w_hh.T)
    gibt = np.stack([
        b_ih[0:H] + b_hh[0:H],
        b_ih[H:2 * H] + b_hh[H:2 * H],
        b_ih[2 * H:],
        b_hh[2 * H:],
    ], axis=0).astype(np.float32)
    bcol_v = np.stack([
        b_hh[2 * H:],                          # t1 stt scalar (b_hh_n)
        -b_ih[2 * H:],                         # tanh bias (-b_ih_n)
        b_ih[H:2 * H] + b_hh[H:2 * H],         # zn-sweep sigmoid bias (b_z)
    ], axis=1).astype(np.float32)
    in_maps = []
    for c in range(NCORES):
        idx = perm[:, c]                       # slot p -> original seq index
        in_maps.append({
            "xt": np.ascontiguousarray(xt_all[idx]),
            "wih3": wih3, "whh3": whh3, "gibt": gibt, "bcol": bcol_v,
            "onesd": np.ones((1, KMAX), np.float32),
        })
    return in_maps


LAST_RESULTS = None


def kernel(x, seq_len, w_ih, w_hh, b_ih, b_hh):
    global LAST_RESULTS
    from concourse import bass_utils
    T = x.shape[1]
    perm, plans = _assignment(seq_len, T)
    key = (T, plans)
    if key not in _CACHE:
        _CACHE[key] = _build(T, plans)
    nc = _CACHE[key]
    in_maps = _host_prep(np.asarray(x), np.asarray(seq_len), np.asarray(w_ih),
                         np.asarray(w_hh), np.asarray(b_ih), np.asarray(b_hh),
                         perm)
    res = bass_utils.run_bass_kernel_spmd(nc, in_maps,
                                          core_ids=list(range(NCORES)))
    LAST_RESULTS = res
    y = np.empty((B, T, H), np.float32)
    for c in range(NCORES):
        ytc = np.array(res.results[c]["yt"])   # [BC, H, T]
        for p in range(BC):
            t0, K = plans[p][-1]
            t_end = t0 + K
            if t_end < T:
                # past the slot's plan end, h is frozen: replicate last col
                ytc[p, :, t_end:] = ytc[p, :, t_end - 1][:, None]
        y[perm[:, c]] = ytc.transpose(0, 2, 1)
    return np.ascontiguousarray(y)


# revision 51
# speedup vs baseline: 1.1475x; 1.0521x over previous
"""GRU (ragged sequences) Trainium2 Bass kernel — chunked-Picard v2b.

The GRU is solved per time-chunk by Picard iteration (the step map is
strongly contractive), with the h-recurrence solved exactly along the
chunk by one tensor_tensor_scan per sweep:

  sweep s (gates from the previous iterate's trajectory, wide over t):
    s_g  = W_g_hh h_prev[t-1] + W_g_ih x_t + b_g    (PE, f32r, psum accum)
    r, z = sigmoid(s_rz)                            (Act)
    pre  = s_n_ih + r * (W_n_hh h_prev[t-1] + bhn)  (DVE stt + PE accum)
    n    = tanh(pre)                                (Act)
    h_t  = z_t h_{t-1} + (1-z_t) n_t                (exact affine scan, DVE)

v2b vs v1:
  * gi is RECOMPUTED on PE each sweep (Wih x accumulated into the same
    psum group as Whh h) instead of precomputed + evacuated to SBUF:
    kills all three PSUM->SBUF evacuation ops per chunk (DVE was the
    bottleneck engine) at the cost of PE matmuls (PE has headroom).
  * Sweep schedule (rzn, zn, rzn, zn): the r gate is only recomputed on
    sweeps 0 and 2 (rel err 9.9e-3 vs 7.9e-3 for full, budget 2e-2).
  * Variable-width chunk plans per slot: the last chunk of each slot is
    trimmed to the slot's max sequence length (rounded up to 64, min 256
    to keep f32r matmuls at 1 cycle/row): 23 -> 20.5 chunk-equivalents.
  * Ragged masking via host-side x poisoning: for t >= seq_len, x[:,t]
    is replaced by v solving W_z_ih v + b_ih_z = 40, so z saturates to
    exactly 1.0 in fp32 and h freezes bit-exactly.  Kills the mask row
    DMA and the per-chunk mask matmul.
  * Output tail (t >= slot plan end) filled on host from the last column
    instead of on-device broadcast+DMA.

Sequences are sorted by length and interleaved across cores (core c gets
ranks c, c+8, ...) so all cores share one live pattern / one program.
x is host-pretransposed to [B, I, T]; output is [B, H, T].
"""

import sys
import numpy as np

sys.path.insert(0, "/opt/trn_rl_repo")

B, T_FULL, I, H = 64, 2048, 128, 128
NCORES = 8
BC = B // NCORES          # sequences per core
KMAX = 512
SCHED = ("rzn", "zn", "rzn", "zn")
USE_SWEEP0 = False   # per-partition gh0 sweep-0 form: measured slower (queue hops)

_CACHE = {}


def _plan_slot(maxlen, T):
    """ceil(maxlen/512) chunks of near-equal width covering exactly
    max(maxlen, 256*n) columns — every width lands in [256, 512] so f32r
    matmuls stay at 1 cycle/row and no column is processed needlessly."""
    n = max(1, -(-maxlen // KMAX))
    total = min(T, max(maxlen, 256 * n))
    q = -(-total // 4)                  # distribute in 4-col units: odd
    plan = []                           # matmul widths fail the ISA check
    t0 = 0
    for i in range(n):
        w = 4 * (q // n + (1 if i < q % n else 0))
        w = min(w, T - t0)
        plan.append((t0, w))
        t0 += w
    return tuple(plan)


def _assignment(seq_len, T):
    """Interleaved sorted assignment: core c, slot p <- rank p*NCORES + c."""
    sl = np.asarray(seq_len)
    order = np.argsort(-sl, kind="stable")
    perm = order.reshape(BC, NCORES)           # [slot, core]
    plans = tuple(_plan_slot(int(sl[perm[p]].max()), T) for p in range(BC))
    return perm, plans


def _chains(plans, G=6):
    """Pack the BC slot-groups into G serial chains with near-equal chunk
    counts (greedy LPT) so every round keeps >=G-1 independent chains alive
    (the tail rounds of the plain per-slot layout starve the engines).
    h resets to zero at group boundaries inside a chain.
    Returns: list of chains; each chain is a list of
    (group p, t0, K, is_group_start) chunk records."""
    order = sorted(range(len(plans)), key=lambda p: -len(plans[p]))
    chains = [[] for _ in range(G)]
    loads = [0] * G
    for p in order:
        i = loads.index(min(loads))
        chains[i].append(p)
        loads[i] += len(plans[p])
    out = []
    for groups in chains:
        recs = []
        for p in groups:
            for k, (t0, K) in enumerate(plans[p]):
                recs.append((p, t0, K, k == 0))
        out.append(recs)
    # longest chains first so the final round's chains start earliest
    out.sort(key=len, reverse=True)
    return out


def _build(T, plans):
    from contextlib import ExitStack
    import concourse.bacc as bacc
    import concourse.mybir as mybir
    import concourse.tile as tile

    f32 = mybir.dt.float32
    f32r = mybir.dt.float32r
    Alu = mybir.AluOpType
    Act = mybir.ActivationFunctionType

    nc = bacc.Bacc("TRN2", target_bir_lowering=False, debug=False,
                   num_devices=NCORES)

    xt = nc.dram_tensor("xt", [BC, I, T], f32r, kind="ExternalInput").ap()
    wih3 = nc.dram_tensor("wih3", [I, 3 * H], f32r, kind="ExternalInput").ap()
    whh3 = nc.dram_tensor("whh3", [H, 3 * H], f32r, kind="ExternalInput").ap()
    # per-gate total biases as 1-row weights: r,z: b_ih+b_hh, n: b_ih only,
    # row 3: b_hh_n (for the sweep-0 gh0 trick)
    gibt = nc.dram_tensor("gibt", [4, 128], f32r, kind="ExternalInput").ap()
    # bias cols: 0: b_hh_n (t1 scalar), 1: -b_ih_n (tanh bias), 2: b_z total
    bcol = nc.dram_tensor("bcol", [H, 3], f32, kind="ExternalInput").ap()
    onesd = nc.dram_tensor("onesd", [1, KMAX], f32r, kind="ExternalInput").ap()
    yt = nc.dram_tensor("yt", [BC, H, T], f32r, kind="ExternalOutput").ap()

    import os
    NB = lambda k, d: int(os.environ.get(k, d))
    with tile.TileContext(nc) as tc, ExitStack() as ctx:
        const = ctx.enter_context(tc.tile_pool(name="const", bufs=1))
        xpool = ctx.enter_context(tc.tile_pool(name="x", bufs=NB("XB", 2)))
        hppool = ctx.enter_context(tc.tile_pool(name="hp", bufs=NB("HB", 2)))
        rzpool = ctx.enter_context(tc.tile_pool(name="rz", bufs=NB("RB", 1)))
        npool = ctx.enter_context(tc.tile_pool(name="nn", bufs=NB("NN", 8)))
        unpool = ctx.enter_context(tc.tile_pool(name="un", bufs=NB("NN", 8)))
        zcpool = ctx.enter_context(tc.tile_pool(name="zc", bufs=NB("NN", 8)))
        ghpool = ctx.enter_context(tc.tile_pool(name="gh0", bufs=2))
        ps_rz = ctx.enter_context(tc.tile_pool(name="ps_rz", bufs=NB("PB", 4),
                                               space="PSUM"))

        wih_sb = const.tile([128, 3 * H], f32r, tag="wih")
        nc.sync.dma_start(out=wih_sb[:], in_=wih3)
        whh_sb = const.tile([128, 3 * H], f32r, tag="whh")
        nc.sync.dma_start(out=whh_sb[:], in_=whh3)
        gib_rows = []
        for g in range(4):
            row = const.tile([1, 128], f32r, tag=f"gib{g}", name=f"gib{g}")
            nc.sync.dma_start(out=row[:], in_=gibt[g:g + 1, :])
            gib_rows.append(row)
        bcol_sb = const.tile([128, 3], f32, tag="bcol")
        nc.sync.dma_start(out=bcol_sb[:], in_=bcol)
        ones_sb = const.tile([1, KMAX], f32r, tag="ones")
        nc.sync.dma_start(out=ones_sb[:], in_=onesd)
        zero_e = const.tile([128, 2], f32, tag="zeroe")
        nc.vector.memset(zero_e[:], 0.0)
        zero_er = const.tile([128, 2], f32r, tag="zeroer")
        nc.vector.tensor_copy(out=zero_er[:], in_=zero_e[:])
        brc_sb = const.tile([128, KMAX], f32, tag="brc")
        nc.vector.memset(brc_sb[:], 0.0)

        # entry: f32 view (scan initial / scalar operands) + f32r 2-col view
        # (matmul data; 1-col matmuls fail the ISA check) of the previous
        # chunk's final h
        entry = {b: zero_e[:, 0:1] for b in range(BC)}
        entry_r = {b: zero_er[:, 0:2] for b in range(BC)}
        hps, xs, rzs = {}, {}, {}

        def preamble(b, p, t0, K, reset):
            if reset:
                # first chunk of a new sequence-group in this chain
                entry[b] = zero_e[:, 0:1]
                entry_r[b] = zero_er[:, 0:2]
            xtile = xpool.tile([128, KMAX], f32r, tag=f"x{b}", name=f"x{b}")
            nc.sync.dma_start(out=xtile[:, 0:K], in_=xt[p, :, t0:t0 + K])
            xs[b] = xtile
            # hp trajectory tile: col 0 = h_entry, cols 1..K = h_1..h_K.
            hp = hppool.tile([128, KMAX + 1], f32r, tag=f"hp{b}", name=f"hp{b}")
            if USE_SWEEP0:
                # sweep-0 gh0 form needs no broadcast; only col 0 must hold
                # the entry for later sweeps' matmuls.
                nc.gpsimd.tensor_copy(out=hp[:, 0:1], in_=entry[b])
            else:
                # sweep-0 guess: h_prev[t] = h_entry for all t (brc as zero
                # shape-donor: no false dep); alternate engine by slot.
                import os
                hpe = os.environ.get("HPE", "par")
                on_pool = {"par": b % 2 == 0, "pool": True,
                           "dve": False}[hpe]
                eng = nc.gpsimd if on_pool else nc.vector
                eng.tensor_scalar(out=hp[:, 0:K], in0=brc_sb[:, 0:K],
                                  scalar1=0.0, scalar2=entry[b],
                                  op0=Alu.mult, op1=Alu.add)
            hps[b] = hp
            rzs[b] = rzpool.tile([128, 2 * KMAX], f32, tag=f"rz{b}",
                                 name=f"rz{b}")

        def sweep0(b, t0, K, un_on_pool):
            """Sweep 0: the trajectory guess is the constant h_entry, so
            W_hh h collapses to per-partition scalars gh0 = W_hh h_entry + b
            computed by six 1-column matmuls; the wide matmuls are only the
            three W_ih x products and the gate biases ride activation bias
            APs / the stt scalar."""
            hp, xtile, rz = hps[b], xs[b], rzs[b]
            prz = ps_rz.tile([128, 2 * KMAX], f32, tag="przn")
            Z0 = KMAX
            # gh0: 2-col matmuls (1-col matmuls fail the ISA check); only
            # the second output column of each pair is meaningful.
            # pair g: col 2g+1 = W_g e + bias_g  (g=2 bias is b_hh_n)
            for g in range(3):
                nc.tensor.matmul(prz[:, 2 * g:2 * g + 2],
                                 whh_sb[:, g * 128:(g + 1) * 128],
                                 entry_r[b], start=True, stop=False)
                nc.tensor.matmul(prz[:, 2 * g:2 * g + 2],
                                 gib_rows[g if g < 2 else 3][0:1, :],
                                 ones_sb[0:1, 0:2], start=False, stop=True)
            gh0 = ghpool.tile([128, 6], f32, tag="gh0", name="gh0")
            nc.vector.tensor_copy(out=gh0[:], in_=prz[:, 0:6])
            # wide input projections (gi) — overwrite the pe0 columns
            nc.tensor.matmul(prz[:, 0:K], wih_sb[:, 0:128], xtile[:, 0:K],
                             start=True, stop=True, skip_group_check=True)
            nc.tensor.matmul(prz[:, Z0:Z0 + K], wih_sb[:, 128:256],
                             xtile[:, 0:K], start=True, stop=True)
            nc.scalar.activation(rz[:, 0:K], prz[:, 0:K], Act.Sigmoid,
                                 bias=gh0[:, 1:2])
            nc.scalar.activation(rz[:, Z0:Z0 + K], prz[:, Z0:Z0 + K],
                                 Act.Sigmoid, bias=gh0[:, 3:4])
            # gi_n into the (dead) r region
            nc.tensor.matmul(prz[:, 0:K], wih_sb[:, 256:384], xtile[:, 0:K],
                             start=True, stop=True, skip_group_check=True)
            # pre' = r * gh0_n + gi_n  (b_ih_n rides the tanh bias)
            nc.vector.scalar_tensor_tensor(
                out=prz[:, Z0:Z0 + K], in0=rz[:, 0:K], scalar=gh0[:, 5:6],
                in1=prz[:, 0:K], op0=Alu.mult, op1=Alu.add)
            nsb = npool.tile([128, KMAX], f32, tag="nn", name="nsb")
            nc.scalar.activation(nsb[:, 0:K], prz[:, Z0:Z0 + K],
                                 Act.Tanh, scale=-1.0, bias=bcol_sb[:, 1:2])
            un = unpool.tile([128, KMAX], f32, tag="un", name="un")
            if un_on_pool:
                zc = zcpool.tile([128, KMAX], f32, tag="zc", name="zc")
                nc.gpsimd.tensor_scalar(out=zc[:, 0:K],
                                        in0=rz[:, Z0:Z0 + K], scalar1=1.0,
                                        scalar2=None, op0=Alu.subtract)
                nc.gpsimd.tensor_tensor(out=un[:, 0:K], in0=zc[:, 0:K],
                                        in1=nsb[:, 0:K], op=Alu.mult)
            else:
                nc.vector.scalar_tensor_tensor(
                    out=un[:, 0:K], in0=rz[:, Z0:Z0 + K], scalar=1.0,
                    in1=nsb[:, 0:K], op0=Alu.subtract, op1=Alu.mult)
            nc.vector.tensor_tensor_scan(
                out=hp[:, 1:K + 1], data0=rz[:, Z0:Z0 + K],
                data1=un[:, 0:K], initial=entry[b],
                op0=Alu.mult, op1=Alu.add)

        def sweep(b, gates, un_on_pool, prz, of, w, init_ap):
            """One sweep over columns [of, of+w) of slot b's current chunk.
            Splitting a sweep into column pieces (thin rounds) shortens the
            serial chain: stage s of piece j overlaps stage s+1 of piece
            j-1.  The scan chains across pieces via init_ap."""
            hp, xtile, rz = hps[b], xs[b], rzs[b]
            Z0 = KMAX
            lo, hi = of, of + w          # r region / trajectory window
            zlo, zhi = Z0 + of, Z0 + of + w   # z region window
            if "r" in gates:
                # r group with the bias as a 1-row matmul (merged rz sigmoid
                # can't take per-gate bias APs)
                nc.tensor.matmul(prz[:, lo:hi], whh_sb[:, 0:128],
                                 hp[:, lo:hi], start=True, stop=False)
                nc.tensor.matmul(prz[:, lo:hi], wih_sb[:, 0:128],
                                 xtile[:, lo:hi], start=False, stop=False)
                nc.tensor.matmul(prz[:, lo:hi], gib_rows[0][0:1, :],
                                 ones_sb[0:1, 0:w], start=False, stop=True)
                nc.tensor.matmul(prz[:, zlo:zhi], whh_sb[:, 128:256],
                                 hp[:, lo:hi], start=True, stop=False)
                nc.tensor.matmul(prz[:, zlo:zhi], wih_sb[:, 128:256],
                                 xtile[:, lo:hi], start=False, stop=False)
                nc.tensor.matmul(prz[:, zlo:zhi], gib_rows[1][0:1, :],
                                 ones_sb[0:1, 0:w], start=False, stop=True)
                prz3 = prz.rearrange("p (g k) -> p g k", g=2)
                rz3 = rz.rearrange("p (g k) -> p g k", g=2)
                nc.scalar.activation(rz3[:, :, lo:hi], prz3[:, :, lo:hi],
                                     Act.Sigmoid)
            else:
                # z-only: bias rides the sigmoid's per-partition bias AP
                nc.tensor.matmul(prz[:, zlo:zhi], whh_sb[:, 128:256],
                                 hp[:, lo:hi], start=True, stop=False)
                nc.tensor.matmul(prz[:, zlo:zhi], wih_sb[:, 128:256],
                                 xtile[:, lo:hi], start=False, stop=True)
                nc.scalar.activation(rz[:, zlo:zhi], prz[:, zlo:zhi],
                                     Act.Sigmoid, bias=bcol_sb[:, 2:3])
            # ghn into the (dead or unused) r psum region
            nc.tensor.matmul(prz[:, lo:hi], whh_sb[:, 256:384], hp[:, lo:hi],
                             start=True, stop=True, skip_group_check=True)
            # t1 = (ghn + bhn) * r  -> overwrite dead s_z psum region
            nc.vector.scalar_tensor_tensor(
                out=prz[:, zlo:zhi], in0=prz[:, lo:hi],
                scalar=bcol_sb[:, 0:1],
                in1=rz[:, lo:hi], op0=Alu.add, op1=Alu.mult)
            # pre' = t1 + W_n_ih x: PE accumulates onto t1 in-place (psum
            # has_written bits from the s_z matmuls survive the DVE
            # overwrite, so start=False adds).  b_ih_n rides the tanh bias.
            nc.tensor.matmul(prz[:, zlo:zhi], wih_sb[:, 256:384],
                             xtile[:, lo:hi], start=False, stop=True,
                             skip_group_check=True)
            # nneg = tanh(-(pre' + b_ih_n)) = -n  (negation via scale,
            # b_ih_n via the per-partition bias AP: bias col 1 = -b_ih_n)
            nsb = npool.tile([128, KMAX], f32, tag="nn", name="nsb")
            nc.scalar.activation(nsb[:, 0:w], prz[:, zlo:zhi],
                                 Act.Tanh, scale=-1.0, bias=bcol_sb[:, 1:2])
            # un = (z-1)*(-n) = (1-z)*n
            un = unpool.tile([128, KMAX], f32, tag="un", name="un")
            if un_on_pool:
                # GpSimd path (SBUF-only): zc = z-1, then un = zc * nneg
                zc = zcpool.tile([128, KMAX], f32, tag="zc", name="zc")
                nc.gpsimd.tensor_scalar(out=zc[:, 0:w],
                                        in0=rz[:, zlo:zhi], scalar1=1.0,
                                        scalar2=None, op0=Alu.subtract)
                nc.gpsimd.tensor_tensor(out=un[:, 0:w], in0=zc[:, 0:w],
                                        in1=nsb[:, 0:w], op=Alu.mult)
            else:
                nc.vector.scalar_tensor_tensor(
                    out=un[:, 0:w], in0=rz[:, zlo:zhi], scalar=1.0,
                    in1=nsb[:, 0:w], op0=Alu.subtract, op1=Alu.mult)
            # exact affine solve along the piece: h_t = z_t h_{t-1} + un_t
            nc.vector.tensor_tensor_scan(
                out=hp[:, lo + 1:hi + 1], data0=rz[:, zlo:zhi],
                data1=un[:, 0:w], initial=init_ap,
                op0=Alu.mult, op1=Alu.add)

        def finish(b, p, t0, K):
            hp = hps[b]
            nc.sync.dma_start(out=yt[p, :, t0:t0 + K], in_=hp[:, 1:K + 1])
            entry[b] = hp[:, K:K + 1].bitcast(f32)
            entry_r[b] = hp[:, K - 1:K + 1]

        import os
        chains = _chains(plans, G=int(os.environ.get("KG", "6")))
        nrounds = max(len(c) for c in chains)
        for ci in range(nrounds):
            livebs = [b for b in range(len(chains)) if ci < len(chains[b])]
            for b in livebs:
                p, t0, K, reset = chains[b][ci]
                preamble(b, p, t0, K, reset)
            unpol = os.environ.get("UNPOL", "par")
            nl = len(livebs)
            S = len(SCHED)

            def emit(b, bi, s):
                p, t0, K, reset = chains[b][ci]
                gates = SCHED[s]
                # spread un across DVE and Pool inside every sweep phase
                # so neither engine becomes the phase bottleneck
                if unpol == "none":
                    unp = False
                elif unpol == "par":
                    unp = nl >= 5 and (b + s) % 2 == 0
                else:  # "kXY": X of nl on pool in full sweeps, Y in zn
                    kf, kz = int(unpol[1]), int(unpol[2])
                    k = kf if "r" in gates else kz
                    unp = nl >= 5 and ((bi + s * 3) % nl) < k
                prz = ps_rz.tile([128, 2 * KMAX], f32, tag="przn",
                                 name="przn")
                sweep(b, gates, unp, prz, 0, K, entry[b])
                if s == S - 1:
                    finish(b, p, t0, K)

            if os.environ.get("ORD", "fwd") == "diag":
                # software-pipelined diagonal: chain bi runs sweep w-bi in
                # wave w, mixing Act-heavy (full) and DVE-heavy (zn) sweeps
                # in every engine queue window
                for w in range(S + nl - 1):
                    for bi, b in enumerate(livebs):
                        s = w - bi
                        if 0 <= s < S:
                            emit(b, bi, s)
            else:
                for s in range(S):
                    for bi, b in enumerate(livebs):
                        emit(b, bi, s)

    nc.compile()
    return nc


def _host_prep(x, seq_len, w_ih, w_hh, b_ih, b_hh, perm):
    T = x.shape[1]
    x = np.asarray(x, np.float32)
    w_ih = np.asarray(w_ih, np.float32)
    w_hh = np.asarray(w_hh, np.float32)
    b_ih = np.asarray(b_ih, np.float32)
    b_hh = np.asarray(b_hh, np.float32)
    seq_len = np.asarray(seq_len).astype(np.int64)
    xt_all = np.ascontiguousarray(x.transpose(0, 2, 1))  # [B, I, T]
    # Poison columns t >= seq_len so that gi_z + b_ih_z ~= 60: z saturates
    # to exactly 1.0 in fp32 (gh_z is bounded by ~6) and h freezes
    # bit-exactly, reproducing the reference's frozen outputs past seq_len.
    # Truncated-SVD solve: tiny singular directions of W_z_ih are dropped so
    # that ||v|| stays small enough for the PE's reduced-precision f32r
    # accumulation (a full solve can give ||v|| ~ 1e6 on an ill-conditioned
    # W_z and f32r noise ~1e3 destroys the freeze).  Dropping sigma_i only
    # perturbs s_z by ~ +-c|u_i^T 1||u_i| << c, still far above saturation.
    Wz = w_ih[H:2 * H].astype(np.float64)
    c = np.full(H, 60.0) - b_ih[H:2 * H].astype(np.float64)
    U, S, Vt = np.linalg.svd(Wz)
    Sinv = np.where(S >= S.max() / 300.0, 1.0 / S, 0.0)
    v = (Vt.T @ (Sinv * (U.T @ c))).astype(np.float32)
    for b in range(B):
        if seq_len[b] < T:
            xt_all[b, :, seq_len[b]:] = v[:, None]
    wih3 = np.ascontiguousarray(w_ih.T)
    whh3 = np.ascontiguousarray(w_hh.T)
    gibt = np.stack([
        b_ih[0:H] + b_hh[0:H],
        b_ih[H:2 * H] + b_hh[H:2 * H],
        b_ih[2 * H:],
        b_hh[2 * H:],
    ], axis=0).astype(np.float32)
    bcol_v = np.stack([
        b_hh[2 * H:],                          # t1 stt scalar (b_hh_n)
        -b_ih[2 * H:],                         # tanh bias (-b_ih_n)
        b_ih[H:2 * H] + b_hh[H:2 * H],         # zn-sweep sigmoid bias (b_z)
    ], axis=1).astype(np.float32)
    in_maps = []
    for c in range(NCORES):
        idx = perm[:, c]                       # slot p -> original seq index
        in_maps.append({
            "xt": np.ascontiguousarray(xt_all[idx]),
            "wih3": wih3, "whh3": whh3, "gibt": gibt, "bcol": bcol_v,
            "onesd": np.ones((1, KMAX), np.float32),
        })
    return in_maps


LAST_RESULTS = None


def kernel(x, seq_len, w_ih, w_hh, b_ih, b_hh):
    global LAST_RESULTS
    from concourse import bass_utils
    T = x.shape[1]
    perm, plans = _assignment(seq_len, T)
    key = (T, plans)
    if key not in _CACHE:
        _CACHE[key] = _build(T, plans)
    nc = _CACHE[key]
    in_maps = _host_prep(np.asarray(x), np.asarray(seq_len), np.asarray(w_ih),
                         np.asarray(w_hh), np.asarray(b_ih), np.asarray(b_hh),
                         perm)
    res = bass_utils.run_bass_kernel_spmd(nc, in_maps,
                                          core_ids=list(range(NCORES)))
    LAST_RESULTS = res
    y = np.empty((B, T, H), np.float32)
    for c in range(NCORES):
        ytc = np.array(res.results[c]["yt"])   # [BC, H, T]
        for p in range(BC):
            t0, K = plans[p][-1]
            t_end = t0 + K
            if t_end < T:
                # past the slot's plan end, h is frozen: replicate last col
                ytc[p, :, t_end:] = ytc[p, :, t_end - 1][:, None]
        y[perm[:, c]] = ytc.transpose(0, 2, 1)
    return np.ascontiguousarray(y)


# revision 54
# speedup vs baseline: 1.1671x; 1.0170x over previous
"""GRU (ragged sequences) Trainium2 Bass kernel — chunked-Picard v2b.

The GRU is solved per time-chunk by Picard iteration (the step map is
strongly contractive), with the h-recurrence solved exactly along the
chunk by one tensor_tensor_scan per sweep:

  sweep s (gates from the previous iterate's trajectory, wide over t):
    s_g  = W_g_hh h_prev[t-1] + W_g_ih x_t + b_g    (PE, f32r, psum accum)
    r, z = sigmoid(s_rz)                            (Act)
    pre  = s_n_ih + r * (W_n_hh h_prev[t-1] + bhn)  (DVE stt + PE accum)
    n    = tanh(pre)                                (Act)
    h_t  = z_t h_{t-1} + (1-z_t) n_t                (exact affine scan, DVE)

v2b vs v1:
  * gi is RECOMPUTED on PE each sweep (Wih x accumulated into the same
    psum group as Whh h) instead of precomputed + evacuated to SBUF:
    kills all three PSUM->SBUF evacuation ops per chunk (DVE was the
    bottleneck engine) at the cost of PE matmuls (PE has headroom).
  * Sweep schedule (rzn, zn, rzn, zn): the r gate is only recomputed on
    sweeps 0 and 2 (rel err 9.9e-3 vs 7.9e-3 for full, budget 2e-2).
  * Variable-width chunk plans per slot: the last chunk of each slot is
    trimmed to the slot's max sequence length (rounded up to 64, min 256
    to keep f32r matmuls at 1 cycle/row): 23 -> 20.5 chunk-equivalents.
  * Ragged masking via host-side x poisoning: for t >= seq_len, x[:,t]
    is replaced by v solving W_z_ih v + b_ih_z = 40, so z saturates to
    exactly 1.0 in fp32 and h freezes bit-exactly.  Kills the mask row
    DMA and the per-chunk mask matmul.
  * Output tail (t >= slot plan end) filled on host from the last column
    instead of on-device broadcast+DMA.

Sequences are sorted by length and interleaved across cores (core c gets
ranks c, c+8, ...) so all cores share one live pattern / one program.
x is host-pretransposed to [B, I, T]; output is [B, H, T].
"""

import sys
import numpy as np

sys.path.insert(0, "/opt/trn_rl_repo")

B, T_FULL, I, H = 64, 2048, 128, 128
NCORES = 8
BC = B // NCORES          # sequences per core
KMAX = 512
SCHED = ("rzn", "zn", "rzn", "zn")


_CACHE = {}


def _plan_slot(maxlen, T):
    """ceil(maxlen/512) chunks of near-equal width covering exactly
    max(maxlen, 256*n) columns — every width lands in [256, 512] so f32r
    matmuls stay at 1 cycle/row and no column is processed needlessly."""
    n = max(1, -(-maxlen // KMAX))
    total = min(T, max(maxlen, 256 * n))
    q = -(-total // 4)                  # distribute in 4-col units: odd
    plan = []                           # matmul widths fail the ISA check
    t0 = 0
    for i in range(n):
        w = 4 * (q // n + (1 if i < q % n else 0))
        w = min(w, T - t0)
        plan.append((t0, w))
        t0 += w
    return tuple(plan)


def _assignment(seq_len, T):
    """Interleaved sorted assignment: core c, slot p <- rank p*NCORES + c."""
    sl = np.asarray(seq_len)
    order = np.argsort(-sl, kind="stable")
    perm = order.reshape(BC, NCORES)           # [slot, core]
    plans = tuple(_plan_slot(int(sl[perm[p]].max()), T) for p in range(BC))
    return perm, plans


def _chains(plans, G=6):
    """Pack the BC slot-groups into G serial chains with near-equal chunk
    counts (greedy LPT) so every round keeps >=G-1 independent chains alive
    (the tail rounds of the plain per-slot layout starve the engines).
    h resets to zero at group boundaries inside a chain.
    Returns: list of chains; each chain is a list of
    (group p, t0, K, is_group_start) chunk records."""
    order = sorted(range(len(plans)), key=lambda p: -len(plans[p]))
    chains = [[] for _ in range(G)]
    loads = [0] * G
    for p in order:
        i = loads.index(min(loads))
        chains[i].append(p)
        loads[i] += len(plans[p])
    out = []
    for groups in chains:
        recs = []
        for p in groups:
            for k, (t0, K) in enumerate(plans[p]):
                recs.append((p, t0, K, k == 0))
        out.append(recs)
    # longest chains first so the final round's chains start earliest
    out.sort(key=len, reverse=True)
    return out


def _build(T, plans):
    from contextlib import ExitStack
    import concourse.bacc as bacc
    import concourse.mybir as mybir
    import concourse.tile as tile

    f32 = mybir.dt.float32
    f32r = mybir.dt.float32r
    Alu = mybir.AluOpType
    Act = mybir.ActivationFunctionType

    nc = bacc.Bacc("TRN2", target_bir_lowering=False, debug=False,
                   num_devices=NCORES)

    xt = nc.dram_tensor("xt", [BC, I, T], f32r, kind="ExternalInput").ap()
    wih3 = nc.dram_tensor("wih3", [I, 3 * H], f32r, kind="ExternalInput").ap()
    whh3 = nc.dram_tensor("whh3", [H, 3 * H], f32r, kind="ExternalInput").ap()
    # per-gate total biases as 1-row weights: r,z: b_ih+b_hh, n: b_ih only,
    # row 3: b_hh_n (for the sweep-0 gh0 trick)
    gibt = nc.dram_tensor("gibt", [4, 128], f32r, kind="ExternalInput").ap()
    # bias cols: 0: b_hh_n (t1 scalar), 1: -b_ih_n (tanh bias), 2: b_z total
    bcol = nc.dram_tensor("bcol", [H, 3], f32, kind="ExternalInput").ap()
    onesd = nc.dram_tensor("onesd", [1, KMAX], f32r, kind="ExternalInput").ap()
    yt = nc.dram_tensor("yt", [BC, H, T], f32r, kind="ExternalOutput").ap()

    import os
    NB = lambda k, d: int(os.environ.get(k, d))
    USE_SWEEP0 = os.environ.get("SW0", "1") == "1"  # per-partition gh0 sweep-0 form
    with tile.TileContext(nc) as tc, ExitStack() as ctx:
        const = ctx.enter_context(tc.tile_pool(name="const", bufs=1))
        xpool = ctx.enter_context(tc.tile_pool(name="x", bufs=NB("XB", 2)))
        hppool = ctx.enter_context(tc.tile_pool(name="hp", bufs=NB("HB", 2)))
        rzpool = ctx.enter_context(tc.tile_pool(name="rz", bufs=NB("RB", 1)))
        npool = ctx.enter_context(tc.tile_pool(name="nn", bufs=NB("NN", 8)))
        unpool = ctx.enter_context(tc.tile_pool(name="un", bufs=NB("NN", 8)))
        zcpool = ctx.enter_context(tc.tile_pool(name="zc", bufs=NB("NN", 8)))
        ghpool = ctx.enter_context(tc.tile_pool(name="gh0", bufs=2))
        ps_rz = ctx.enter_context(tc.tile_pool(name="ps_rz", bufs=NB("PB", 4),
                                               space="PSUM"))

        wih_sb = const.tile([128, 3 * H], f32r, tag="wih")
        nc.sync.dma_start(out=wih_sb[:], in_=wih3)
        whh_sb = const.tile([128, 3 * H], f32r, tag="whh")
        nc.sync.dma_start(out=whh_sb[:], in_=whh3)
        gib_rows = []
        for g in range(4):
            row = const.tile([1, 128], f32r, tag=f"gib{g}", name=f"gib{g}")
            nc.sync.dma_start(out=row[:], in_=gibt[g:g + 1, :])
            gib_rows.append(row)
        bcol_sb = const.tile([128, 3], f32, tag="bcol")
        nc.sync.dma_start(out=bcol_sb[:], in_=bcol)
        ones_sb = const.tile([1, KMAX], f32r, tag="ones")
        nc.sync.dma_start(out=ones_sb[:], in_=onesd)
        zero_e = const.tile([128, 2], f32, tag="zeroe")
        nc.vector.memset(zero_e[:], 0.0)
        zero_er = const.tile([128, 2], f32r, tag="zeroer")
        nc.vector.tensor_copy(out=zero_er[:], in_=zero_e[:])
        brc_sb = const.tile([128, KMAX], f32, tag="brc")
        nc.vector.memset(brc_sb[:], 0.0)

        # entry: f32 view (scan initial / scalar operands) + f32r 2-col view
        # (matmul data; 1-col matmuls fail the ISA check) of the previous
        # chunk's final h
        entry = {b: zero_e[:, 0:1] for b in range(BC)}
        entry_r = {b: zero_er[:, 0:2] for b in range(BC)}
        hps, xs, rzs = {}, {}, {}

        def preamble(b, p, t0, K, reset):
            if reset:
                # first chunk of a new sequence-group in this chain
                entry[b] = zero_e[:, 0:1]
                entry_r[b] = zero_er[:, 0:2]
            xtile = xpool.tile([128, KMAX], f32r, tag=f"x{b}", name=f"x{b}")
            nc.sync.dma_start(out=xtile[:, 0:K], in_=xt[p, :, t0:t0 + K])
            xs[b] = xtile
            # hp trajectory tile: col 0 = h_entry, cols 1..K = h_1..h_K.
            hp = hppool.tile([128, KMAX + 1], f32r, tag=f"hp{b}", name=f"hp{b}")
            if USE_SWEEP0:
                # sweep-0 gh0 form needs no broadcast; only col 0 must hold
                # the entry for later sweeps' matmuls.
                nc.gpsimd.tensor_copy(out=hp[:, 0:1], in_=entry[b])
            else:
                # sweep-0 guess: h_prev[t] = h_entry for all t (brc as zero
                # shape-donor: no false dep); alternate engine by slot.
                import os
                hpe = os.environ.get("HPE", "par")
                on_pool = {"par": b % 2 == 0, "pool": True,
                           "dve": False}[hpe]
                eng = nc.gpsimd if on_pool else nc.vector
                eng.tensor_scalar(out=hp[:, 0:K], in0=brc_sb[:, 0:K],
                                  scalar1=0.0, scalar2=entry[b],
                                  op0=Alu.mult, op1=Alu.add)
            hps[b] = hp
            rzs[b] = rzpool.tile([128, 2 * KMAX], f32, tag=f"rz{b}",
                                 name=f"rz{b}")

        def sweep0(b, t0, K, un_on_pool):
            """Sweep 0: the trajectory guess is the constant h_entry, so
            W_hh h collapses to per-partition scalars gh0 = W_hh h_entry + b
            computed by six 1-column matmuls; the wide matmuls are only the
            three W_ih x products and the gate biases ride activation bias
            APs / the stt scalar."""
            hp, xtile, rz = hps[b], xs[b], rzs[b]
            prz = ps_rz.tile([128, 2 * KMAX], f32, tag="przn")
            Z0 = KMAX
            # gh0: 2-col matmuls (1-col matmuls fail the ISA check); only
            # the second output column of each pair is meaningful.
            # pair g: col 2g+1 = W_g e + bias_g  (g=2 bias is b_hh_n)
            for g in range(3):
                nc.tensor.matmul(prz[:, 2 * g:2 * g + 2],
                                 whh_sb[:, g * 128:(g + 1) * 128],
                                 entry_r[b], start=True, stop=False)
                nc.tensor.matmul(prz[:, 2 * g:2 * g + 2],
                                 gib_rows[g if g < 2 else 3][0:1, :],
                                 ones_sb[0:1, 0:2], start=False, stop=True)
            gh0 = ghpool.tile([128, 6], f32, tag="gh0", name="gh0")
            nc.vector.tensor_copy(out=gh0[:], in_=prz[:, 0:6])
            # wide input projections (gi) — overwrite the pe0 columns
            nc.tensor.matmul(prz[:, 0:K], wih_sb[:, 0:128], xtile[:, 0:K],
                             start=True, stop=True, skip_group_check=True)
            nc.tensor.matmul(prz[:, Z0:Z0 + K], wih_sb[:, 128:256],
                             xtile[:, 0:K], start=True, stop=True)
            nc.scalar.activation(rz[:, 0:K], prz[:, 0:K], Act.Sigmoid,
                                 bias=gh0[:, 1:2])
            nc.scalar.activation(rz[:, Z0:Z0 + K], prz[:, Z0:Z0 + K],
                                 Act.Sigmoid, bias=gh0[:, 3:4])
            # gi_n into the (dead) r region
            nc.tensor.matmul(prz[:, 0:K], wih_sb[:, 256:384], xtile[:, 0:K],
                             start=True, stop=True, skip_group_check=True)
            # pre' = r * gh0_n + gi_n  (b_ih_n rides the tanh bias)
            nc.vector.scalar_tensor_tensor(
                out=prz[:, Z0:Z0 + K], in0=rz[:, 0:K], scalar=gh0[:, 5:6],
                in1=prz[:, 0:K], op0=Alu.mult, op1=Alu.add)
            nsb = npool.tile([128, KMAX], f32, tag="nn", name="nsb")
            nc.scalar.activation(nsb[:, 0:K], prz[:, Z0:Z0 + K],
                                 Act.Tanh, scale=-1.0, bias=bcol_sb[:, 1:2])
            un = unpool.tile([128, KMAX], f32, tag="un", name="un")
            if un_on_pool:
                zc = zcpool.tile([128, KMAX], f32, tag="zc", name="zc")
                nc.gpsimd.tensor_scalar(out=zc[:, 0:K],
                                        in0=rz[:, Z0:Z0 + K], scalar1=1.0,
                                        scalar2=None, op0=Alu.subtract)
                nc.gpsimd.tensor_tensor(out=un[:, 0:K], in0=zc[:, 0:K],
                                        in1=nsb[:, 0:K], op=Alu.mult)
            else:
                nc.vector.scalar_tensor_tensor(
                    out=un[:, 0:K], in0=rz[:, Z0:Z0 + K], scalar=1.0,
                    in1=nsb[:, 0:K], op0=Alu.subtract, op1=Alu.mult)
            nc.vector.tensor_tensor_scan(
                out=hp[:, 1:K + 1], data0=rz[:, Z0:Z0 + K],
                data1=un[:, 0:K], initial=entry[b],
                op0=Alu.mult, op1=Alu.add)

        def sweep(b, gates, un_on_pool, prz, of, w, init_ap):
            """One sweep over columns [of, of+w) of slot b's current chunk.
            Splitting a sweep into column pieces (thin rounds) shortens the
            serial chain: stage s of piece j overlaps stage s+1 of piece
            j-1.  The scan chains across pieces via init_ap."""
            hp, xtile, rz = hps[b], xs[b], rzs[b]
            Z0 = KMAX
            lo, hi = of, of + w          # r region / trajectory window
            zlo, zhi = Z0 + of, Z0 + of + w   # z region window
            if "r" in gates:
                # r group with the bias as a 1-row matmul (merged rz sigmoid
                # can't take per-gate bias APs)
                nc.tensor.matmul(prz[:, lo:hi], whh_sb[:, 0:128],
                                 hp[:, lo:hi], start=True, stop=False)
                nc.tensor.matmul(prz[:, lo:hi], wih_sb[:, 0:128],
                                 xtile[:, lo:hi], start=False, stop=False)
                nc.tensor.matmul(prz[:, lo:hi], gib_rows[0][0:1, :],
                                 ones_sb[0:1, 0:w], start=False, stop=True)
                nc.tensor.matmul(prz[:, zlo:zhi], whh_sb[:, 128:256],
                                 hp[:, lo:hi], start=True, stop=False)
                nc.tensor.matmul(prz[:, zlo:zhi], wih_sb[:, 128:256],
                                 xtile[:, lo:hi], start=False, stop=False)
                nc.tensor.matmul(prz[:, zlo:zhi], gib_rows[1][0:1, :],
                                 ones_sb[0:1, 0:w], start=False, stop=True)
                prz3 = prz.rearrange("p (g k) -> p g k", g=2)
                rz3 = rz.rearrange("p (g k) -> p g k", g=2)
                nc.scalar.activation(rz3[:, :, lo:hi], prz3[:, :, lo:hi],
                                     Act.Sigmoid)
            else:
                # z-only: bias rides the sigmoid's per-partition bias AP
                nc.tensor.matmul(prz[:, zlo:zhi], whh_sb[:, 128:256],
                                 hp[:, lo:hi], start=True, stop=False)
                nc.tensor.matmul(prz[:, zlo:zhi], wih_sb[:, 128:256],
                                 xtile[:, lo:hi], start=False, stop=True)
                nc.scalar.activation(rz[:, zlo:zhi], prz[:, zlo:zhi],
                                     Act.Sigmoid, bias=bcol_sb[:, 2:3])
            # ghn into the (dead or unused) r psum region
            nc.tensor.matmul(prz[:, lo:hi], whh_sb[:, 256:384], hp[:, lo:hi],
                             start=True, stop=True, skip_group_check=True)
            # t1 = (ghn + bhn) * r  -> overwrite dead s_z psum region
            nc.vector.scalar_tensor_tensor(
                out=prz[:, zlo:zhi], in0=prz[:, lo:hi],
                scalar=bcol_sb[:, 0:1],
                in1=rz[:, lo:hi], op0=Alu.add, op1=Alu.mult)
            # pre' = t1 + W_n_ih x: PE accumulates onto t1 in-place (psum
            # has_written bits from the s_z matmuls survive the DVE
            # overwrite, so start=False adds).  b_ih_n rides the tanh bias.
            nc.tensor.matmul(prz[:, zlo:zhi], wih_sb[:, 256:384],
                             xtile[:, lo:hi], start=False, stop=True,
                             skip_group_check=True)
            # nneg = tanh(-(pre' + b_ih_n)) = -n  (negation via scale,
            # b_ih_n via the per-partition bias AP: bias col 1 = -b_ih_n)
            nsb = npool.tile([128, KMAX], f32, tag="nn", name="nsb")
            nc.scalar.activation(nsb[:, 0:w], prz[:, zlo:zhi],
                                 Act.Tanh, scale=-1.0, bias=bcol_sb[:, 1:2])
            # un = (z-1)*(-n) = (1-z)*n
            un = unpool.tile([128, KMAX], f32, tag="un", name="un")
            if un_on_pool:
                # GpSimd path (SBUF-only): zc = z-1, then un = zc * nneg
                zc = zcpool.tile([128, KMAX], f32, tag="zc", name="zc")
                nc.gpsimd.tensor_scalar(out=zc[:, 0:w],
                                        in0=rz[:, zlo:zhi], scalar1=1.0,
                                        scalar2=None, op0=Alu.subtract)
                nc.gpsimd.tensor_tensor(out=un[:, 0:w], in0=zc[:, 0:w],
                                        in1=nsb[:, 0:w], op=Alu.mult)
            else:
                nc.vector.scalar_tensor_tensor(
                    out=un[:, 0:w], in0=rz[:, zlo:zhi], scalar=1.0,
                    in1=nsb[:, 0:w], op0=Alu.subtract, op1=Alu.mult)
            # exact affine solve along the piece: h_t = z_t h_{t-1} + un_t
            nc.vector.tensor_tensor_scan(
                out=hp[:, lo + 1:hi + 1], data0=rz[:, zlo:zhi],
                data1=un[:, 0:w], initial=init_ap,
                op0=Alu.mult, op1=Alu.add)

        def finish(b, p, t0, K):
            hp = hps[b]
            nc.sync.dma_start(out=yt[p, :, t0:t0 + K], in_=hp[:, 1:K + 1])
            entry[b] = hp[:, K:K + 1].bitcast(f32)
            entry_r[b] = hp[:, K - 1:K + 1]

        import os
        chains = _chains(plans, G=int(os.environ.get("KG", "7")))
        nrounds = max(len(c) for c in chains)
        for ci in range(nrounds):
            livebs = [b for b in range(len(chains)) if ci < len(chains[b])]
            for b in livebs:
                p, t0, K, reset = chains[b][ci]
                preamble(b, p, t0, K, reset)
            unpol = os.environ.get("UNPOL", "par")
            nl = len(livebs)
            S = len(SCHED)

            def emit(b, bi, s):
                p, t0, K, reset = chains[b][ci]
                gates = SCHED[s]
                # spread un across DVE and Pool inside every sweep phase
                # so neither engine becomes the phase bottleneck
                if unpol == "none":
                    unp = False
                elif unpol == "par":
                    unp = nl >= 5 and (b + s) % 2 == 0
                else:  # "kXY": X of nl on pool in full sweeps, Y in zn
                    kf, kz = int(unpol[1]), int(unpol[2])
                    k = kf if "r" in gates else kz
                    unp = nl >= 5 and ((bi + s * 3) % nl) < k
                prz = ps_rz.tile([128, 2 * KMAX], f32, tag="przn",
                                 name="przn")
                sweep(b, gates, unp, prz, 0, K, entry[b])
                if s == S - 1:
                    finish(b, p, t0, K)

            if os.environ.get("ORD", "fwd") == "diag":
                # software-pipelined diagonal: chain bi runs sweep w-bi in
                # wave w, mixing Act-heavy (full) and DVE-heavy (zn) sweeps
                # in every engine queue window
                for w in range(S + nl - 1):
                    for bi, b in enumerate(livebs):
                        s = w - bi
                        if 0 <= s < S:
                            emit(b, bi, s)
            else:
                for s in range(S):
                    for bi, b in enumerate(livebs):
                        emit(b, bi, s)

    nc.compile()
    return nc


def _host_prep(x, seq_len, w_ih, w_hh, b_ih, b_hh, perm):
    T = x.shape[1]
    x = np.asarray(x, np.float32)
    w_ih = np.asarray(w_ih, np.float32)
    w_hh = np.asarray(w_hh, np.float32)
    b_ih = np.asarray(b_ih, np.float32)
    b_hh = np.asarray(b_hh, np.float32)
    seq_len = np.asarray(seq_len).astype(np.int64)
    xt_all = np.ascontiguousarray(x.transpose(0, 2, 1))  # [B, I, T]
    # Poison columns t >= seq_len so that gi_z + b_ih_z ~= 60: z saturates
    # to exactly 1.0 in fp32 (gh_z is bounded by ~6) and h freezes
    # bit-exactly, reproducing the reference's frozen outputs past seq_len.
    # Truncated-SVD solve: tiny singular directions of W_z_ih are dropped so
    # that ||v|| stays small enough for the PE's reduced-precision f32r
    # accumulation (a full solve can give ||v|| ~ 1e6 on an ill-conditioned
    # W_z and f32r noise ~1e3 destroys the freeze).  Dropping sigma_i only
    # perturbs s_z by ~ +-c|u_i^T 1||u_i| << c, still far above saturation.
    Wz = w_ih[H:2 * H].astype(np.float64)
    c = np.full(H, 60.0) - b_ih[H:2 * H].astype(np.float64)
    U, S, Vt = np.linalg.svd(Wz)
    Sinv = np.where(S >= S.max() / 300.0, 1.0 / S, 0.0)
    v = (Vt.T @ (Sinv * (U.T @ c))).astype(np.float32)
    for b in range(B):
        if seq_len[b] < T:
            xt_all[b, :, seq_len[b]:] = v[:, None]
    wih3 = np.ascontiguousarray(w_ih.T)
    whh3 = np.ascontiguousarray(w_hh.T)
    gibt = np.stack([
        b_ih[0:H] + b_hh[0:H],
        b_ih[H:2 * H] + b_hh[H:2 * H],
        b_ih[2 * H:],
        b_hh[2 * H:],
    ], axis=0).astype(np.float32)
    bcol_v = np.stack([
        b_hh[2 * H:],                          # t1 stt scalar (b_hh_n)
        -b_ih[2 * H:],                         # tanh bias (-b_ih_n)
        b_ih[H:2 * H] + b_hh[H:2 * H],         # zn-sweep sigmoid bias (b_z)
    ], axis=1).astype(np.float32)
    in_maps = []
    for c in range(NCORES):
        idx = perm[:, c]                       # slot p -> original seq index
        in_maps.append({
            "xt": np.ascontiguousarray(xt_all[idx]),
            "wih3": wih3, "whh3": whh3, "gibt": gibt, "bcol": bcol_v,
            "onesd": np.ones((1, KMAX), np.float32),
        })
    return in_maps


LAST_RESULTS = None


def kernel(x, seq_len, w_ih, w_hh, b_ih, b_hh):
    global LAST_RESULTS
    from concourse import bass_utils
    T = x.shape[1]
    perm, plans = _assignment(seq_len, T)
    key = (T, plans)
    if key not in _CACHE:
        _CACHE[key] = _build(T, plans)
    nc = _CACHE[key]
    in_maps = _host_prep(np.asarray(x), np.asarray(seq_len), np.asarray(w_ih),
                         np.asarray(w_hh), np.asarray(b_ih), np.asarray(b_hh),
                         perm)
    res = bass_utils.run_bass_kernel_spmd(nc, in_maps,
                                          core_ids=list(range(NCORES)))
    LAST_RESULTS = res
    y = np.empty((B, T, H), np.float32)
    for c in range(NCORES):
        ytc = np.array(res.results[c]["yt"])   # [BC, H, T]
        for p in range(BC):
            t0, K = plans[p][-1]
            t_end = t0 + K
            if t_end < T:
                # past the slot's plan end, h is frozen: replicate last col
                ytc[p, :, t_end:] = ytc[p, :, t_end - 1][:, None]
        y[perm[:, c]] = ytc.transpose(0, 2, 1)
    return np.ascontiguousarray(y)
